# revision 44
# baseline (speedup 1.0000x reference)
"""Trainium2 Bass kernel for a 3-layer GraphSAGE GNN (mean aggregation) +
global_add_pool + 2-layer MLP head, distributed over 8 NeuronCores.

Sharding: nodes are split into 8 contiguous slabs (by dst); each core owns the
edges whose dst lands in its slab.  Aggregation is gather + one-hot matmul
segment-sum (no dma_scatter_add, no serialization):

  - Host sorts each core's edges by (src-chunk, dst-block) and pads each
    (block, chunk) group to a fixed t_bc tiles of 128 edges, so the device
    program is data-independent and identical across cores.
  - Per layer: dma_gather streams h[src] rows (256B bf16) from a replicated
    node-major HBM table on 4 SWDGE queues (no inter-gather deps).
  - A [128 edge, 128 dst, 8 tile] one-hot matrix built on DVE (is_equal
    against an iota) turns per-block segment-sum into 16 accumulating
    matmuls into a PSUM tile.
  - Dense phase per 128-node block: mean scale (DVE), transpose (PE),
    h = relu(mean @ Wl + [h_prev|1] @ [Wr;b]) with bias folded into the
    root matmul via an appended ones row.
  - The next-layer table is replicated in TWO half-slab AllGathers into
    Shared-address-space DRAM halves (half-major layout): the first half
    ships while the second half of the blocks is still computing, and the
    next layer's gathers on chunks 0-1 start before the second collective
    lands.
Pooling by graph id via one-hot matmuls, AllReduce, tiny MLP head.
"""

import numpy as np

import concourse.bass as bass
import concourse.mybir as mybir
import concourse.tile as tile
from concourse import bacc

F32 = mybir.dt.float32
BF16 = mybir.dt.bfloat16
I16 = mybir.dt.int16

# ---------------------------------------------------------------- config

N_NODES = 100000
N_EDGES = 1200000
N_GRAPHS = 256
D_IN = 8
D_H = 64
N_CORES = 8

REAL = N_NODES // N_CORES          # 12500 real nodes per core
SLAB = ((REAL + 127) // 128) * 128  # 12544
NBLK = SLAB // 128                 # 98 dst blocks per core
HALF_ROWS = (NBLK // 2) * 128      # 6272 rows per slab half
HALF_TBL = N_CORES * HALF_ROWS     # 50176 rows per table half
TBL_ROWS = 2 * HALF_TBL            # 100352
CHUNK = 2 * HALF_ROWS * 2          # 25088 table rows per index chunk (int16)
NCH = 4                            # chunks
# Table layout is half-major: row = half*HALF_TBL + core*HALF_ROWS + s, so
# each half is a separate Shared tensor written by exactly one AllGather.
PADROW = 0                         # any finite row: one-hot drops pad edges
PAD_DLOC = 999.0                   # one-hot miss value for pad edges


class Cfg:
    """T = 128-row tiles per gather instruction.  The SWDGE descriptor ring
    holds 128 descriptors per DMA engine; a gather of num_idxs needs
    num_idxs/16 + 1 slots, so num_idxs must stay <= 1920 (T <= 15) or the
    ucode deadlocks in await_space.  T must also be a multiple of t_bc."""

    def __init__(self, t_bc):
        assert t_bc in (1, 2, 4, 8)
        self.t_bc = t_bc                      # tiles per (block, chunk)
        # 1024-idx gathers: each DMA engine's 64 descriptors fill exactly one
        # 16KB SDMA packet (single_packet mode caps num_idxs at 1024 for 256B
        # rows -- larger wedges the device).  512-idx gathers measured slower
        # (per-instruction overhead), so T=8 is the sweet spot.
        self.T = 8                            # tiles per gather instruction
        self.bpi = self.T // t_bc             # blocks per instruction
        self.ipb = (NBLK * t_bc + self.T - 1) // self.T  # gather insts/bucket
        self.bt = self.ipb * self.T           # tiles per bucket (padded)
        self.n_inst = NCH * self.ipb          # gather insts per layer


# ---------------------------------------------------------------- host prep

def _wrap_idx(a):
    """[n_inst, G] int -> [128, n_inst, G//16] wrapped-16 + replicated."""
    n_inst, g = a.shape
    a = a.reshape(n_inst, g // 16, 16).transpose(0, 2, 1)   # [n_inst, 16, g//16]
    a = np.tile(a, (1, 8, 1))                               # [n_inst, 128, g//16]
    return np.ascontiguousarray(a.transpose(1, 0, 2)).astype(np.int16)


def build_host_data(x, edge_index, batch):
    import ml_dtypes
    bf = ml_dtypes.bfloat16
    x = np.asarray(x, np.float32)
    src = np.asarray(edge_index[0], np.int64)
    dst = np.asarray(edge_index[1], np.int64)
    batch = np.asarray(batch, np.int64)

    deg = np.bincount(dst, minlength=N_NODES).astype(np.float64)
    invc_full = (1.0 / np.maximum(deg, 1.0)).astype(np.float32)

    core_of = dst // REAL
    dloc_all = dst - core_of * REAL
    e_blk = dloc_all // 128
    e_dib = (dloc_all % 128).astype(np.float32)
    s_core = src // REAL
    s_loc = src % REAL
    s_half = (s_loc >= HALF_ROWS).astype(np.int64)
    e_ch = s_half * 2 + s_core // 4
    e_cidx = (s_core % 4) * HALF_ROWS + (s_loc - s_half * HALF_ROWS)

    cnt = np.zeros((N_CORES, NBLK, NCH), np.int64)
    np.add.at(cnt, (core_of, e_blk, e_ch), 1)
    need = int(-(-cnt.max() // 128))
    t_bc = 1
    while t_bc < need:
        t_bc *= 2
    assert t_bc <= 8, cnt.max()
    cfg = Cfg(t_bc)

    # Trailing-pad skip: positions past the last real edge of the LAST block
    # group in each gather instruction are set to -1 (the SWDGE ucode pops
    # trailing negatives and generates no descriptors for them).  The cut
    # M must be uniform across cores (one SPMD program: num_idxs_reg ==
    # count of valid idx on every core), so M = last-group offset + the
    # max count over cores, padded with valid PADROW rows up to M.
    gsz = cfg.t_bc * 128                      # idx span of one block group
    gpb = cfg.bpi                             # groups (blocks) per instruction
    # count per (core, ch, inst) of the last group in that instruction
    last_blk = np.minimum((np.arange(cfg.ipb) + 1) * gpb - 1, NBLK - 1)
    c_last = cnt[:, last_blk, :]              # [core, ipb, NCH]
    m_inst = (gpb - 1) * gsz + c_last.max(axis=0).T   # [NCH, ipb]
    cfg.m_inst = np.maximum(m_inst, 1).astype(np.int64)

    # xpad table: half-major bf16, 128-wide rows (first 8 cols real)
    xpad = np.zeros((TBL_ROWS, 128), bf)
    n = np.arange(N_NODES)
    n_core, n_loc = n // REAL, n % REAL
    n_half = (n_loc >= HALF_ROWS).astype(np.int64)
    rows = (n_half * HALF_TBL + n_core * HALF_ROWS
            + (n_loc - n_half * HALF_ROWS))
    xpad[rows, :D_IN] = x.astype(bf)

    # iotaT[p, d*T + t] = d  (for one-hot build, O8T layout [128, 128, T])
    iotaT = np.tile(np.arange(128, dtype=np.float32)[None, :, None],
                    (128, 1, cfg.T)).reshape(128, 128 * cfg.T).astype(bf)
    # iota256 for pooling one-hot
    iota256 = np.tile(np.arange(256, dtype=np.float32)[None, :],
                      (128, 1)).astype(bf)
    identf = np.eye(128, dtype=np.float32)
    padmask = (np.arange(128) < (REAL % 128 or 128)).astype(np.float32)
    padmask = padmask.reshape(128, 1)

    shared = dict(xpad=xpad, iotaT=iotaT, iota256=iota256,
                  identf=identf, padmask=padmask)

    per_core = []
    for c in range(N_CORES):
        sel = core_of == c
        s_ch = e_ch[sel]
        s_blk = e_blk[sel]
        s_dib = e_dib[sel]
        s_cidx = e_cidx[sel]
        order = np.lexsort((s_blk, s_ch))
        s_ch, s_blk, s_dib, s_cidx = (s_ch[order], s_blk[order],
                                      s_dib[order], s_cidx[order])
        key = s_ch * NBLK + s_blk
        # rank within each (ch, blk) group (key is sorted)
        starts = np.r_[0, np.flatnonzero(np.diff(key)) + 1]
        group_id = np.cumsum(np.r_[0, np.diff(key) != 0])
        rank = np.arange(len(key)) - starts[group_id]
        assert rank.max() < cfg.t_bc * 128

        tile_j = s_blk * cfg.t_bc + rank // 128   # tile within bucket
        lane = rank % 128
        inst_b = tile_j // cfg.T
        slot = tile_j % cfg.T
        gi = s_ch * cfg.ipb + inst_b
        gpos = slot * 128 + lane

        garr = np.full((cfg.n_inst, cfg.T * 128), PADROW, np.int64)
        garr[gi, gpos] = s_cidx
        m_flat = cfg.m_inst.reshape(-1)       # [n_inst], ch-major like gi
        pos = np.arange(cfg.T * 128)[None, :]
        garr[pos >= m_flat[:, None]] = -1     # trailing pads: no descriptors
        dloc = np.full((128, NCH * cfg.ipb, cfg.T), PAD_DLOC, np.float32)
        dloc[lane, gi, slot] = s_dib

        lo = c * REAL
        loc = np.arange(REAL)
        invc_t = np.zeros((128, NBLK), np.float32)
        invc_t[loc % 128, loc // 128] = invc_full[lo:lo + REAL]
        batch_t = np.full((128, NBLK), -1.0, np.float32)
        batch_t[loc % 128, loc // 128] = batch[lo:lo + REAL].astype(np.float32)

        xfull = np.zeros((128, NBLK, D_IN), np.float32)
        xfull[loc % 128, loc // 128, :] = x[lo:lo + REAL]

        per_core.append(dict(gidx=_wrap_idx(garr), dloc=dloc.astype(bf),
                             invc=invc_t, batchv=batch_t, xfull=xfull))
    return cfg, shared, per_core


def weight_inputs(W1l, b1, W1r, W2l, b2, W2r, W3l, b3, W3r, Wc1, bc1, Wc2, bc2):
    import ml_dtypes
    bf = ml_dtypes.bfloat16
    f = lambda a: np.asarray(a, np.float32)
    le = lambda W, b: np.vstack([f(W), f(b)[None, :]])
    return dict(
        w1le=le(W1l, b1), w2le=le(W2l, b2), w3le=le(W3l, b3),
        w1r=f(W1r), w2r=f(W2r), w3r=f(W3r),
        wc1=f(Wc1), wc2=f(Wc2),
        bc1=f(bc1).reshape(-1, 1),            # [32, 1]
        bc2=f(bc2).reshape(1, 1),
    )


# ---------------------------------------------------------------- device build

def build_gnn(tc, out_ap, ins, cfg):
    nc = tc.nc
    NH = N_GRAPHS // 128              # 2 graph tiles
    Relu = mybir.ActivationFunctionType.Relu
    Copy = mybir.ActivationFunctionType.Copy

    sb = tc.alloc_tile_pool(name="sb", bufs=1)
    msgp = tc.alloc_tile_pool(name="msg", bufs=5)
    o8p = tc.alloc_tile_pool(name="o8", bufs=5)
    hbp = tc.alloc_tile_pool(name="hb", bufs=6)
    tmpp = tc.alloc_tile_pool(name="tmp", bufs=6)
    psA = tc.alloc_tile_pool(name="psA", bufs=2, space="PSUM")
    psR = tc.alloc_tile_pool(name="psR", bufs=2, space="PSUM")
    psT = tc.alloc_tile_pool(name="psT", bufs=1, space="PSUM")
    psG = tc.alloc_tile_pool(name="psG", bufs=1, space="PSUM")
    dram = tc.alloc_tile_pool(name="dram", bufs=1, space="DRAM")

    def load(name, shape, dt=F32):
        t = sb.tile(shape, dt, tag=name)
        nc.sync.dma_start(t[:], ins[name])
        return t

    gidx = load("gidx", [128, cfg.n_inst, cfg.T * 8], I16)
    dloc = load("dloc", [128, NCH * cfg.ipb, cfg.T], BF16)
    iotaT = load("iotaT", [128, 128 * cfg.T], BF16)
    iota256 = load("iota256", [128, 256], BF16)
    identf = load("identf", [128, 128], F32)
    invc = load("invc", [128, NBLK])
    batchv = load("batchv", [128, NBLK])
    padmask = load("padmask", [128, 1])
    xfull = load("xfull", [128, NBLK, D_IN], F32)
    w = {}
    for k in ("w1le", "w2le", "w3le"):
        w[k] = load(k, [(D_IN if k == "w1le" else D_H) + 1, D_H], F32)
    for k in ("w1r", "w2r", "w3r"):
        w[k] = load(k, [D_IN if k == "w1r" else D_H, D_H], F32)
    w["wc1"] = load("wc1", [D_H, D_H // 2])
    w["wc2"] = load("wc2", [D_H // 2, 1])
    w["bc1"] = load("bc1", [D_H // 2, 1])
    w["bc2"] = load("bc2", [1, 1])

    hfull = [sb.tile([128, NBLK, D_H], F32, tag=f"hfull{i}", name=f"hfull{i}")
             for i in range(2)]

    # pre-zero every msg ring buffer: gather positions past num_idxs_reg are
    # skipped (stale SBUF), and uninitialized bits could be NaN — 0 * NaN
    # would poison the one-hot matmul accumulation
    for _ in range(5):
        for ch in range(NCH):
            mz = msgp.tile([128, cfg.T, 128], BF16, tag=f"msg{ch}",
                           name=f"mz{ch}")
            nc.vector.memset(mz[:], 0.0)

    # table halves are Shared so the AllGather writes remote HBM directly;
    # the split also lets the first half's AllGather overlap the second
    # half of the blocks still computing
    H_BLK = NBLK // 2                  # 49 blocks
    tbls = [[dram.tile([HALF_TBL, 128], BF16, name=f"tbl{i}{h}",
                       addr_space="Shared")
             for h in range(2)] for i in range(2)]
    slabs = [[dram.tile([HALF_ROWS, 128], BF16, name=f"slab{i}{h}")
              for h in range(2)] for i in range(2)]
    g_in = dram.tile([N_GRAPHS, D_H], F32)
    g_out = dram.tile([N_GRAPHS, D_H], F32)

    tables = [[ins["xpad"][0:HALF_TBL, :], ins["xpad"][HALF_TBL:TBL_ROWS, :]],
              [tbls[0][0][:], tbls[0][1][:]],
              [tbls[1][0][:], tbls[1][1][:]]]
    pg = [psG.tile([128, D_H], F32, name=f"pg{j}", tag=f"pg{j}")
          for j in range(NH)]

    for layer in range(3):
        tbl_ap = tables[layer]
        Wle = w[("w1le", "w2le", "w3le")[layer]]
        Wr = w[("w1r", "w2r", "w3r")[layer]]
        h_prev = (xfull, hfull[0], hfull[1])[layer]
        kprev = D_IN if layer == 0 else D_H
        h_next = (hfull[0], hfull[1], None)[layer]
        slab_d = (slabs[0], slabs[1], None)[layer]
        kag = D_IN if layer == 0 else D_H

        msgs = {}   # inst -> [msg tiles per ch]
        o8s = {}

        def phase_a(b):
            agg = psA.tile([128, kag], F32, tag="agg", padded_shape=[128, D_H])
            i = (b * cfg.t_bc) // cfg.T
            for ch in range(NCH):
                for k in range(cfg.t_bc):
                    j = b * cfg.t_bc + k
                    slot = j % cfg.T
                    nc.tensor.matmul(
                        agg[:], lhsT=o8s[i][ch][:, :, slot],
                        rhs=msgs[i][ch][:, slot, :kag],
                        start=(ch == 0 and k == 0),
                        stop=(ch == NCH - 1 and k == cfg.t_bc - 1))
            mean_sb = hbp.tile([128, kag], F32, tag="mean")
            nc.vector.tensor_scalar(
                out=mean_sb[:], in0=agg[:], scalar1=invc[:, b:b + 1],
                scalar2=None, op0=mybir.AluOpType.mult)
            tp = psT.tile([kag, 128], F32, tag="tp", padded_shape=[D_H, 128])
            nc.tensor.transpose(tp[:], mean_sb[:], identf[:])
            meanTe = tmpp.tile([kag + 1, 128], F32, tag="meanTe")
            nc.vector.memset(meanTe[:], 1.0)
            nc.scalar.activation(meanTe[:kag, :], tp[:], Copy)
            tpr = psT.tile([kprev, 128], F32, tag="tpr", padded_shape=[D_H, 128])
            nc.tensor.transpose(tpr[:], h_prev[:, b, :kprev], identf[:])
            rootT = tmpp.tile([kprev, 128], F32, tag="rootT")
            nc.scalar.activation(rootT[:], tpr[:], Copy)
            hps = psR.tile([128, D_H], F32, tag="hps")
            nc.tensor.matmul(hps[:], lhsT=meanTe[:], rhs=Wle[:],
                             start=True, stop=False)
            nc.tensor.matmul(hps[:], lhsT=rootT[:],
                             rhs=Wr[:], start=False, stop=True)
            h_bf = hbp.tile([128, D_H], BF16, tag="hbf")
            if layer < 2:
                if b == NBLK - 1:
                    nc.vector.tensor_scalar(
                        out=h_next[:, b, :], in0=hps[:],
                        scalar1=padmask[:, :1], scalar2=0.0,
                        op0=mybir.AluOpType.mult, op1=mybir.AluOpType.max)
                else:
                    nc.vector.tensor_scalar(
                        out=h_next[:, b, :], in0=hps[:],
                        scalar1=0.0, scalar2=None,
                        op0=mybir.AluOpType.max)
                nc.scalar.activation(h_bf[:], h_next[:, b, :], Copy)
                half, brow = (0, b) if b < H_BLK else (1, b - H_BLK)
                nc.sync.dma_start(
                    slab_d[half][:][brow * 128:(brow + 1) * 128, 0:D_H],
                    h_bf[:])
                if b == H_BLK - 1:
                    # first half done: overlap its AllGather with the rest
                    nc.gpsimd.collective_compute(
                        "AllGather", mybir.AluOpType.bypass,
                        replica_groups=[list(range(N_CORES))],
                        ins=[slab_d[0][:]], outs=[tbls[layer][0][:]])
            else:
                nc.vector.tensor_copy(h_bf[:], hps[:])
                gt = tmpp.tile([128, NH * 128], BF16, tag="gt")
                nc.vector.tensor_scalar(
                    out=gt[:], in0=iota256[:], scalar1=batchv[:, b:b + 1],
                    scalar2=None, op0=mybir.AluOpType.is_equal)
                for j in range(NH):
                    nc.tensor.matmul(pg[j][:],
                                     lhsT=gt[:, j * 128:(j + 1) * 128],
                                     rhs=h_bf[:],
                                     start=(b == 0), stop=(b == NBLK - 1))

        for i in range(cfg.ipb + 2):
            if i < cfg.ipb:
                ms, os_ = [], []
                for ch in range(NCH):
                    m = msgp.tile([128, cfg.T, 128], BF16, tag=f"msg{ch}")
                    chunk_ap = tbl_ap[ch // 2][
                        (ch % 2) * CHUNK:(ch % 2 + 1) * CHUNK, :]
                    nc.gpsimd.dma_gather(
                        out_ap=m[:], in_ap=chunk_ap,
                        idxs_ap=gidx[:, ch * cfg.ipb + i, :],
                        num_idxs=cfg.T * 128,
                        num_idxs_reg=int(cfg.m_inst[ch, i]),
                        elem_size=128, queue_num=ch)
                    ms.append(m)
                    o8 = o8p.tile([128, 128, cfg.T], BF16, tag=f"o8{ch}")
                    nc.vector.tensor_tensor(
                        out=o8[:],
                        in0=dloc[:, ch * cfg.ipb + i:ch * cfg.ipb + i + 1, :]
                            .to_broadcast([128, 128, cfg.T]),
                        in1=iotaT[:].rearrange("p (d t) -> p d t", t=cfg.T),
                        op=mybir.AluOpType.is_equal)
                    os_.append(o8)
                msgs[i] = ms
                o8s[i] = os_
            if 1 <= i <= cfg.ipb:
                for bb in range(cfg.bpi):
                    b = (i - 1) * cfg.bpi + bb
                    if b < NBLK:
                        phase_a(b)
            if i >= 2:
                msgs.pop(i - 2, None)
                o8s.pop(i - 2, None)

        if layer < 2:
            nc.gpsimd.collective_compute(
                "AllGather", mybir.AluOpType.bypass,
                replica_groups=[list(range(N_CORES))],
                ins=[slab_d[1][:]], outs=[tbls[layer][1][:]])

    # ---- pooling partials -> AllReduce
    gpart = sb.tile([128, NH, D_H], F32, tag="gpart")
    for j in range(NH):
        nc.vector.tensor_copy(gpart[:, j, :], pg[j][:])
    nc.sync.dma_start(g_in[:].rearrange("(q p) f -> p q f", p=128), gpart[:])
    nc.gpsimd.collective_compute(
        "AllReduce", mybir.AluOpType.add,
        replica_groups=[list(range(N_CORES))],
        ins=[g_in[:]], outs=[g_out[:]])

    # ---- MLP head
    g_sb = sb.tile([128, NH, D_H], F32, tag="gsb")
    nc.sync.dma_start(g_sb[:], g_out[:].rearrange("(q p) f -> p q f", p=128))
    gT = sb.tile([D_H, NH * 128], F32, tag="gT")
    for j in range(NH):
        tp = psG.tile([D_H, 128], F32, tag="pg0")
        nc.tensor.transpose(tp[:], g_sb[:, j, :], identf[:])
        nc.vector.tensor_copy(gT[:, j * 128:(j + 1) * 128], tp[:])
    DC = D_H // 2
    mlp1 = psG.tile([DC, NH * 128], F32, tag="pg0")
    nc.tensor.matmul(mlp1[:], lhsT=w["wc1"][:], rhs=gT[:], start=True, stop=True)
    z = sb.tile([DC, NH * 128], F32, tag="z")
    nc.scalar.activation(z[:], mlp1[:], Relu, bias=w["bc1"][:])
    mlp2 = psG.tile([1, NH * 128], F32, tag="pg1")
    nc.tensor.matmul(mlp2[:], lhsT=w["wc2"][:], rhs=z[:], start=True, stop=True)
    o_sb = sb.tile([1, NH * 128], F32, tag="osb")
    nc.vector.tensor_scalar(out=o_sb[:], in0=mlp2[:], scalar1=w["bc2"][:],
                            scalar2=None, op0=mybir.AluOpType.add)
    nc.sync.dma_start(out_ap.rearrange("a b -> b a"), o_sb[:])

    for p in (dram, psG, psT, psR, psA, tmpp, hbp, o8p, msgp, sb):
        p.release()


# ---------------------------------------------------------------- compile+run

_CACHE = {}


def _compile(cfg):
    key = ("nc", cfg.t_bc, cfg.m_inst.tobytes())
    if key in _CACHE:
        return _CACHE[key]
    nc = bacc.Bacc("TRN2", target_bir_lowering=False, debug=False,
                   num_devices=N_CORES, num_swdge_queues=4)
    shapes = dict(
        xpad=([TBL_ROWS, 128], BF16),
        gidx=([128, cfg.n_inst, cfg.T * 8], I16),
        dloc=([128, NCH * cfg.ipb, cfg.T], BF16),
        iotaT=([128, 128 * cfg.T], BF16),
        iota256=([128, 256], BF16),
        identf=([128, 128], F32),
        invc=([128, NBLK], F32),
        batchv=([128, NBLK], F32),
        padmask=([128, 1], F32),
        xfull=([128, NBLK, D_IN], F32),
        w1le=([D_IN + 1, D_H], F32), w2le=([D_H + 1, D_H], F32),
        w3le=([D_H + 1, D_H], F32),
        w1r=([D_IN, D_H], F32), w2r=([D_H, D_H], F32), w3r=([D_H, D_H], F32),
        wc1=([D_H, D_H // 2], F32), wc2=([D_H // 2, 1], F32),
        bc1=([D_H // 2, 1], F32), bc2=([1, 1], F32),
    )
    ins = {}
    for name, (shp, dt) in shapes.items():
        ins[name] = nc.dram_tensor(name, shp, dt, kind="ExternalInput").ap()
    out = nc.dram_tensor("out", [N_GRAPHS, 1], F32, kind="ExternalOutput")
    with tile.TileContext(nc) as tc:
        build_gnn(tc, out.ap(), ins, cfg)
    nc.compile()
    _CACHE[key] = nc
    return nc


def make_in_maps(inputs):
    cfg, shared, per_core = build_host_data(
        inputs["x"], inputs["edge_index"], inputs["batch"])
    wmap = weight_inputs(
        inputs["W1l"], inputs["b1"], inputs["W1r"], inputs["W2l"], inputs["b2"],
        inputs["W2r"], inputs["W3l"], inputs["b3"], inputs["W3r"],
        inputs["Wc1"], inputs["bc1"], inputs["Wc2"], inputs["bc2"])
    in_maps = []
    for c in range(N_CORES):
        m = {}
        m.update(shared)
        m.update(per_core[c])
        m.update(wmap)
        in_maps.append(m)
    return cfg, in_maps


def _make_executor(nc):
    """Build a reusable jitted 8-core executor for the compiled Bass module."""
    import jax
    from jax.sharding import Mesh, PartitionSpec
    from jax.experimental.shard_map import shard_map
    from concourse.bass2jax import (_bass_exec_p, install_neuronx_cc_hook,
                                    partition_id_tensor)
    install_neuronx_cc_hook()
    partition_name = (nc.partition_id_tensor.name
                      if nc.partition_id_tensor else None)
    in_names, out_names, out_avals = [], [], []
    for alloc in nc.m.functions[0].allocations:
        if not isinstance(alloc, mybir.MemoryLocationSet):
            continue
        name = alloc.memorylocations[0].name
        if alloc.kind == "ExternalInput":
            if name != partition_name:
                in_names.append(name)
        elif alloc.kind == "ExternalOutput":
            out_names.append(name)
            out_avals.append(jax.core.ShapedArray(
                tuple(alloc.tensor_shape), mybir.dt.np(alloc.dtype)))
    n_params = len(in_names)
    in_names_all = list(in_names) + list(out_names)
    if partition_name:
        in_names_all.append(partition_name)

    def _body(*args):
        operands = list(args)
        if partition_name:
            operands.append(partition_id_tensor())
        return tuple(_bass_exec_p.bind(
            *operands, out_avals=tuple(out_avals),
            in_names=tuple(in_names_all), out_names=tuple(out_names),
            lowering_input_output_aliases=(), sim_require_finite=False,
            sim_require_nnan=False, nc=nc))

    devices = jax.devices()[:N_CORES]
    mesh = Mesh(np.asarray(devices), ("core",))
    n_outs = len(out_names)
    sharded = jax.jit(shard_map(
        _body, mesh=mesh,
        in_specs=(PartitionSpec("core"),) * (n_params + n_outs),
        out_specs=(PartitionSpec("core"),) * n_outs, check_rep=False),
        keep_unused=True)

    def run(in_maps):
        concat_in = [np.concatenate([np.asarray(in_maps[c][n])
                                     for c in range(N_CORES)], axis=0)
                     for n in in_names]
        concat_zeros = [np.zeros((N_CORES * a.shape[0], *a.shape[1:]), a.dtype)
                        for a in out_avals]
        args = [jax.device_put(a) for a in concat_in + concat_zeros]
        out_arrs = sharded(*args)
        jax.block_until_ready(out_arrs)
        return {name: np.asarray(out_arrs[i]).reshape(
                    N_CORES, *out_avals[i].shape)[0]
                for i, name in enumerate(out_names)}, (args, sharded)
    return run


def _get_runner(cfg):
    key = ("runner", cfg.t_bc, cfg.m_inst.tobytes())
    if key not in _CACHE:
        _CACHE[key] = _make_executor(_compile(cfg))
    return _CACHE[key]


def kernel(**inputs):
    cfg, in_maps = make_in_maps(inputs)
    run = _get_runner(cfg)
    out, _ = run(in_maps)
    return np.asarray(out["out"], np.float32)



# revision 45
# speedup vs baseline: 1.0496x; 1.0496x over previous
"""Trainium2 Bass kernel for a 3-layer GraphSAGE GNN (mean aggregation) +
global_add_pool + 2-layer MLP head, distributed over 8 NeuronCores.

Sharding: nodes are split into 8 contiguous slabs (by dst); each core owns the
edges whose dst lands in its slab.  Aggregation is gather + one-hot matmul
segment-sum (no dma_scatter_add, no serialization):

  - Host sorts each core's edges by (src-chunk, dst-block) and pads each
    (block, chunk) group to a fixed t_bc tiles of 128 edges, so the device
    program is data-independent and identical across cores.
  - Per layer: dma_gather streams h[src] rows (256B bf16) from a replicated
    node-major HBM table on 4 SWDGE queues (no inter-gather deps).
  - A [128 edge, 128 dst, 8 tile] one-hot matrix built on DVE (is_equal
    against an iota) turns per-block segment-sum into 16 accumulating
    matmuls into a PSUM tile.
  - Dense phase per 128-node block: mean scale (DVE), transpose (PE),
    h = relu(mean @ Wl + [h_prev|1] @ [Wr;b]) with bias folded into the
    root matmul via an appended ones row.
  - The next-layer table is replicated in TWO half-slab AllGathers into
    Shared-address-space DRAM halves (half-major layout): the first half
    ships while the second half of the blocks is still computing, and the
    next layer's gathers on chunks 0-1 start before the second collective
    lands.
Pooling by graph id via one-hot matmuls, AllReduce, tiny MLP head.
"""

import numpy as np

import concourse.bass as bass
import concourse.mybir as mybir
import concourse.tile as tile
from concourse import bacc

F32 = mybir.dt.float32
BF16 = mybir.dt.bfloat16
I16 = mybir.dt.int16

# ---------------------------------------------------------------- config

N_NODES = 100000
N_EDGES = 1200000
N_GRAPHS = 256
D_IN = 8
D_H = 64
N_CORES = 8

REAL = N_NODES // N_CORES          # 12500 real nodes per core
SLAB = ((REAL + 127) // 128) * 128  # 12544
NBLK = SLAB // 128                 # 98 dst blocks per core
HALF_ROWS = (NBLK // 2) * 128      # 6272 rows per slab half
HALF_TBL = N_CORES * HALF_ROWS     # 50176 rows per table half
TBL_ROWS = 2 * HALF_TBL            # 100352
CHUNK = 2 * HALF_ROWS * 2          # 25088 table rows per index chunk (int16)
NCH = 4                            # chunks
# Table layout is half-major: row = half*HALF_TBL + core*HALF_ROWS + s, so
# each half is a separate Shared tensor written by exactly one AllGather.
PADROW = 0                         # any finite row: one-hot drops pad edges
PAD_DLOC = 999.0                   # one-hot miss value for pad edges


class Cfg:
    """T = 128-row tiles per gather instruction.  The SWDGE descriptor ring
    holds 128 descriptors per DMA engine; a gather of num_idxs needs
    num_idxs/16 + 1 slots, so num_idxs must stay <= 1920 (T <= 15) or the
    ucode deadlocks in await_space.  T must also be a multiple of t_bc."""

    def __init__(self, t_bc):
        assert t_bc in (1, 2, 4, 8)
        self.t_bc = t_bc                      # tiles per (block, chunk)
        # 1024-idx gathers: each DMA engine's 64 descriptors fill exactly one
        # 16KB SDMA packet (single_packet mode caps num_idxs at 1024 for 256B
        # rows -- larger wedges the device).  512-idx gathers measured slower
        # (per-instruction overhead), so T=8 is the sweet spot.
        self.T = 8                            # tiles per gather instruction
        self.bpi = self.T // t_bc             # blocks per instruction
        self.ipb = (NBLK * t_bc + self.T - 1) // self.T  # gather insts/bucket
        self.bt = self.ipb * self.T           # tiles per bucket (padded)
        self.n_inst = NCH * self.ipb          # gather insts per layer


# ---------------------------------------------------------------- host prep

def _wrap_idx(a):
    """[n_inst, G] int -> [128, n_inst, G//16] wrapped-16 + replicated."""
    n_inst, g = a.shape
    a = a.reshape(n_inst, g // 16, 16).transpose(0, 2, 1)   # [n_inst, 16, g//16]
    a = np.tile(a, (1, 8, 1))                               # [n_inst, 128, g//16]
    return np.ascontiguousarray(a.transpose(1, 0, 2)).astype(np.int16)


def build_host_data(x, edge_index, batch):
    import ml_dtypes
    bf = ml_dtypes.bfloat16
    x = np.asarray(x, np.float32)
    src = np.asarray(edge_index[0], np.int64)
    dst = np.asarray(edge_index[1], np.int64)
    batch = np.asarray(batch, np.int64)

    deg = np.bincount(dst, minlength=N_NODES).astype(np.float64)
    invc_full = (1.0 / np.maximum(deg, 1.0)).astype(np.float32)

    core_of = dst // REAL
    dloc_all = dst - core_of * REAL
    e_blk = dloc_all // 128
    e_dib = (dloc_all % 128).astype(np.float32)
    s_core = src // REAL
    s_loc = src % REAL
    s_half = (s_loc >= HALF_ROWS).astype(np.int64)
    e_ch = s_half * 2 + s_core // 4
    e_cidx = (s_core % 4) * HALF_ROWS + (s_loc - s_half * HALF_ROWS)

    cnt = np.zeros((N_CORES, NBLK, NCH), np.int64)
    np.add.at(cnt, (core_of, e_blk, e_ch), 1)
    need = int(-(-cnt.max() // 128))
    t_bc = 1
    while t_bc < need:
        t_bc *= 2
    assert t_bc <= 8, cnt.max()
    cfg = Cfg(t_bc)

    # Trailing-pad skip: positions past the last real edge of the LAST block
    # group in each gather instruction are set to -1 (the SWDGE ucode pops
    # trailing negatives and generates no descriptors for them).  The cut
    # M must be uniform across cores (one SPMD program: num_idxs_reg ==
    # count of valid idx on every core), so M = last-group offset + the
    # max count over cores, padded with valid PADROW rows up to M.
    gsz = cfg.t_bc * 128                      # idx span of one block group
    gpb = cfg.bpi                             # groups (blocks) per instruction
    # count per (core, ch, inst) of the last group in that instruction
    last_blk = np.minimum((np.arange(cfg.ipb) + 1) * gpb - 1, NBLK - 1)
    c_last = cnt[:, last_blk, :]              # [core, ipb, NCH]
    m_inst = (gpb - 1) * gsz + c_last.max(axis=0).T   # [NCH, ipb]
    cfg.m_inst = np.maximum(m_inst, 1).astype(np.int64)

    # xpad table: half-major bf16, 128-wide rows (first 8 cols real)
    xpad = np.zeros((TBL_ROWS, 128), bf)
    n = np.arange(N_NODES)
    n_core, n_loc = n // REAL, n % REAL
    n_half = (n_loc >= HALF_ROWS).astype(np.int64)
    rows = (n_half * HALF_TBL + n_core * HALF_ROWS
            + (n_loc - n_half * HALF_ROWS))
    xpad[rows, :D_IN] = x.astype(bf)

    # iotaT[p, d*T + t] = d  (for one-hot build, O8T layout [128, 128, T])
    iotaT = np.tile(np.arange(128, dtype=np.float32)[None, :, None],
                    (128, 1, cfg.T)).reshape(128, 128 * cfg.T).astype(bf)
    # iota256 for pooling one-hot
    iota256 = np.tile(np.arange(256, dtype=np.float32)[None, :],
                      (128, 1)).astype(bf)
    identf = np.eye(128, dtype=np.float32)
    padmask = (np.arange(128) < (REAL % 128 or 128)).astype(np.float32)
    padmask = padmask.reshape(128, 1)

    shared = dict(xpad=xpad, iotaT=iotaT, iota256=iota256,
                  identf=identf, padmask=padmask)

    per_core = []
    for c in range(N_CORES):
        sel = core_of == c
        s_ch = e_ch[sel]
        s_blk = e_blk[sel]
        s_dib = e_dib[sel]
        s_cidx = e_cidx[sel]
        order = np.lexsort((s_blk, s_ch))
        s_ch, s_blk, s_dib, s_cidx = (s_ch[order], s_blk[order],
                                      s_dib[order], s_cidx[order])
        key = s_ch * NBLK + s_blk
        # rank within each (ch, blk) group (key is sorted)
        starts = np.r_[0, np.flatnonzero(np.diff(key)) + 1]
        group_id = np.cumsum(np.r_[0, np.diff(key) != 0])
        rank = np.arange(len(key)) - starts[group_id]
        assert rank.max() < cfg.t_bc * 128

        tile_j = s_blk * cfg.t_bc + rank // 128   # tile within bucket
        lane = rank % 128
        inst_b = tile_j // cfg.T
        slot = tile_j % cfg.T
        gi = s_ch * cfg.ipb + inst_b
        gpos = slot * 128 + lane

        garr = np.full((cfg.n_inst, cfg.T * 128), PADROW, np.int64)
        garr[gi, gpos] = s_cidx
        dloc = np.full((128, NCH * cfg.ipb, cfg.T), PAD_DLOC, np.float32)
        dloc[lane, gi, slot] = s_dib

        lo = c * REAL
        loc = np.arange(REAL)
        invc_t = np.zeros((128, NBLK), np.float32)
        invc_t[loc % 128, loc // 128] = invc_full[lo:lo + REAL]
        batch_t = np.full((128, NBLK), -1.0, np.float32)
        batch_t[loc % 128, loc // 128] = batch[lo:lo + REAL].astype(np.float32)

        xfull = np.zeros((128, NBLK, D_IN), np.float32)
        xfull[loc % 128, loc // 128, :] = x[lo:lo + REAL]

        per_core.append(dict(gidx=_wrap_idx(garr), dloc=dloc.astype(bf),
                             invc=invc_t, batchv=batch_t, xfull=xfull))
    return cfg, shared, per_core


def weight_inputs(W1l, b1, W1r, W2l, b2, W2r, W3l, b3, W3r, Wc1, bc1, Wc2, bc2):
    import ml_dtypes
    bf = ml_dtypes.bfloat16
    f = lambda a: np.asarray(a, np.float32)
    le = lambda W, b: np.vstack([f(W), f(b)[None, :]])
    return dict(
        w1le=le(W1l, b1), w2le=le(W2l, b2), w3le=le(W3l, b3),
        w1r=f(W1r), w2r=f(W2r), w3r=f(W3r),
        wc1=f(Wc1), wc2=f(Wc2),
        bc1=f(bc1).reshape(-1, 1),            # [32, 1]
        bc2=f(bc2).reshape(1, 1),
    )


# ---------------------------------------------------------------- device build

def build_gnn(tc, out_ap, ins, cfg):
    nc = tc.nc
    NH = N_GRAPHS // 128              # 2 graph tiles
    Relu = mybir.ActivationFunctionType.Relu
    Copy = mybir.ActivationFunctionType.Copy

    sb = tc.alloc_tile_pool(name="sb", bufs=1)
    msgp = tc.alloc_tile_pool(name="msg", bufs=5)
    o8p = tc.alloc_tile_pool(name="o8", bufs=5)
    hbp = tc.alloc_tile_pool(name="hb", bufs=6)
    tmpp = tc.alloc_tile_pool(name="tmp", bufs=6)
    psA = tc.alloc_tile_pool(name="psA", bufs=2, space="PSUM")
    psR = tc.alloc_tile_pool(name="psR", bufs=2, space="PSUM")
    psT = tc.alloc_tile_pool(name="psT", bufs=1, space="PSUM")
    psG = tc.alloc_tile_pool(name="psG", bufs=1, space="PSUM")
    dram = tc.alloc_tile_pool(name="dram", bufs=1, space="DRAM")

    def load(name, shape, dt=F32):
        t = sb.tile(shape, dt, tag=name)
        nc.sync.dma_start(t[:], ins[name])
        return t

    gidx = load("gidx", [128, cfg.n_inst, cfg.T * 8], I16)
    dloc = load("dloc", [128, NCH * cfg.ipb, cfg.T], BF16)
    iotaT = load("iotaT", [128, 128 * cfg.T], BF16)
    iota256 = load("iota256", [128, 256], BF16)
    identf = load("identf", [128, 128], F32)
    invc = load("invc", [128, NBLK])
    batchv = load("batchv", [128, NBLK])
    padmask = load("padmask", [128, 1])
    xfull = load("xfull", [128, NBLK, D_IN], F32)
    w = {}
    for k in ("w1le", "w2le", "w3le"):
        w[k] = load(k, [(D_IN if k == "w1le" else D_H) + 1, D_H], F32)
    for k in ("w1r", "w2r", "w3r"):
        w[k] = load(k, [D_IN if k == "w1r" else D_H, D_H], F32)
    w["wc1"] = load("wc1", [D_H, D_H // 2])
    w["wc2"] = load("wc2", [D_H // 2, 1])
    w["bc1"] = load("bc1", [D_H // 2, 1])
    w["bc2"] = load("bc2", [1, 1])

    hfull = [sb.tile([128, NBLK, D_H], F32, tag=f"hfull{i}", name=f"hfull{i}")
             for i in range(2)]

    # table halves are Shared so the AllGather writes remote HBM directly;
    # the split also lets the first half's AllGather overlap the second
    # half of the blocks still computing
    H_BLK = NBLK // 2                  # 49 blocks
    tbls = [[dram.tile([HALF_TBL, 128], BF16, name=f"tbl{i}{h}",
                       addr_space="Shared")
             for h in range(2)] for i in range(2)]
    slabs = [[dram.tile([HALF_ROWS, 128], BF16, name=f"slab{i}{h}")
              for h in range(2)] for i in range(2)]
    g_in = dram.tile([N_GRAPHS, D_H], F32)
    g_out = dram.tile([N_GRAPHS, D_H], F32)

    tables = [[ins["xpad"][0:HALF_TBL, :], ins["xpad"][HALF_TBL:TBL_ROWS, :]],
              [tbls[0][0][:], tbls[0][1][:]],
              [tbls[1][0][:], tbls[1][1][:]]]
    pg = [psG.tile([128, D_H], F32, name=f"pg{j}", tag=f"pg{j}")
          for j in range(NH)]

    for layer in range(3):
        tbl_ap = tables[layer]
        Wle = w[("w1le", "w2le", "w3le")[layer]]
        Wr = w[("w1r", "w2r", "w3r")[layer]]
        h_prev = (xfull, hfull[0], hfull[1])[layer]
        kprev = D_IN if layer == 0 else D_H
        h_next = (hfull[0], hfull[1], None)[layer]
        slab_d = (slabs[0], slabs[1], None)[layer]
        kag = D_IN if layer == 0 else D_H

        msgs = {}   # inst -> [msg tiles per ch]
        o8s = {}

        def phase_a(b):
            agg = psA.tile([128, kag], F32, tag="agg", padded_shape=[128, D_H])
            i = (b * cfg.t_bc) // cfg.T
            for ch in range(NCH):
                for k in range(cfg.t_bc):
                    j = b * cfg.t_bc + k
                    slot = j % cfg.T
                    nc.tensor.matmul(
                        agg[:], lhsT=o8s[i][ch][:, :, slot],
                        rhs=msgs[i][ch][:, slot, :kag],
                        start=(ch == 0 and k == 0),
                        stop=(ch == NCH - 1 and k == cfg.t_bc - 1))
            mean_sb = hbp.tile([128, kag], F32, tag="mean")
            nc.vector.tensor_scalar(
                out=mean_sb[:], in0=agg[:], scalar1=invc[:, b:b + 1],
                scalar2=None, op0=mybir.AluOpType.mult)
            tp = psT.tile([kag, 128], F32, tag="tp", padded_shape=[D_H, 128])
            nc.tensor.transpose(tp[:], mean_sb[:], identf[:])
            meanTe = tmpp.tile([kag + 1, 128], F32, tag="meanTe")
            nc.vector.memset(meanTe[:], 1.0)
            nc.scalar.activation(meanTe[:kag, :], tp[:], Copy)
            tpr = psT.tile([kprev, 128], F32, tag="tpr", padded_shape=[D_H, 128])
            nc.tensor.transpose(tpr[:], h_prev[:, b, :kprev], identf[:])
            rootT = tmpp.tile([kprev, 128], F32, tag="rootT")
            nc.scalar.activation(rootT[:], tpr[:], Copy)
            hps = psR.tile([128, D_H], F32, tag="hps")
            nc.tensor.matmul(hps[:], lhsT=meanTe[:], rhs=Wle[:],
                             start=True, stop=False)
            nc.tensor.matmul(hps[:], lhsT=rootT[:],
                             rhs=Wr[:], start=False, stop=True)
            h_bf = hbp.tile([128, D_H], BF16, tag="hbf")
            if layer < 2:
                if b == NBLK - 1:
                    nc.vector.tensor_scalar(
                        out=h_next[:, b, :], in0=hps[:],
                        scalar1=padmask[:, :1], scalar2=0.0,
                        op0=mybir.AluOpType.mult, op1=mybir.AluOpType.max)
                else:
                    nc.vector.tensor_scalar(
                        out=h_next[:, b, :], in0=hps[:],
                        scalar1=0.0, scalar2=None,
                        op0=mybir.AluOpType.max)
                nc.scalar.activation(h_bf[:], h_next[:, b, :], Copy)
                half, brow = (0, b) if b < H_BLK else (1, b - H_BLK)
                nc.sync.dma_start(
                    slab_d[half][:][brow * 128:(brow + 1) * 128, 0:D_H],
                    h_bf[:])
                if b == H_BLK - 1:
                    # first half done: overlap its AllGather with the rest
                    nc.gpsimd.collective_compute(
                        "AllGather", mybir.AluOpType.bypass,
                        replica_groups=[list(range(N_CORES))],
                        ins=[slab_d[0][:]], outs=[tbls[layer][0][:]])
            else:
                nc.vector.tensor_copy(h_bf[:], hps[:])
                gt = tmpp.tile([128, NH * 128], BF16, tag="gt")
                nc.vector.tensor_scalar(
                    out=gt[:], in0=iota256[:], scalar1=batchv[:, b:b + 1],
                    scalar2=None, op0=mybir.AluOpType.is_equal)
                for j in range(NH):
                    nc.tensor.matmul(pg[j][:],
                                     lhsT=gt[:, j * 128:(j + 1) * 128],
                                     rhs=h_bf[:],
                                     start=(b == 0), stop=(b == NBLK - 1))

        for i in range(cfg.ipb + 2):
            if i < cfg.ipb:
                ms, os_ = [], []
                for ch in range(NCH):
                    m = msgp.tile([128, cfg.T, 128], BF16, tag=f"msg{ch}")
                    chunk_ap = tbl_ap[ch // 2][
                        (ch % 2) * CHUNK:(ch % 2 + 1) * CHUNK, :]
                    nc.gpsimd.dma_gather(
                        out_ap=m[:], in_ap=chunk_ap,
                        idxs_ap=gidx[:, ch * cfg.ipb + i, :],
                        num_idxs=cfg.T * 128, num_idxs_reg=cfg.T * 128,
                        elem_size=128, queue_num=ch)
                    ms.append(m)
                    o8 = o8p.tile([128, 128, cfg.T], BF16, tag=f"o8{ch}")
                    nc.vector.tensor_tensor(
                        out=o8[:],
                        in0=dloc[:, ch * cfg.ipb + i:ch * cfg.ipb + i + 1, :]
                            .to_broadcast([128, 128, cfg.T]),
                        in1=iotaT[:].rearrange("p (d t) -> p d t", t=cfg.T),
                        op=mybir.AluOpType.is_equal)
                    os_.append(o8)
                msgs[i] = ms
                o8s[i] = os_
            if 1 <= i <= cfg.ipb:
                for bb in range(cfg.bpi):
                    b = (i - 1) * cfg.bpi + bb
                    if b < NBLK:
                        phase_a(b)
            if i >= 2:
                msgs.pop(i - 2, None)
                o8s.pop(i - 2, None)

        if layer < 2:
            nc.gpsimd.collective_compute(
                "AllGather", mybir.AluOpType.bypass,
                replica_groups=[list(range(N_CORES))],
                ins=[slab_d[1][:]], outs=[tbls[layer][1][:]])

    # ---- pooling partials -> AllReduce
    gpart = sb.tile([128, NH, D_H], F32, tag="gpart")
    for j in range(NH):
        nc.vector.tensor_copy(gpart[:, j, :], pg[j][:])
    nc.sync.dma_start(g_in[:].rearrange("(q p) f -> p q f", p=128), gpart[:])
    nc.gpsimd.collective_compute(
        "AllReduce", mybir.AluOpType.add,
        replica_groups=[list(range(N_CORES))],
        ins=[g_in[:]], outs=[g_out[:]])

    # ---- MLP head
    g_sb = sb.tile([128, NH, D_H], F32, tag="gsb")
    nc.sync.dma_start(g_sb[:], g_out[:].rearrange("(q p) f -> p q f", p=128))
    gT = sb.tile([D_H, NH * 128], F32, tag="gT")
    for j in range(NH):
        tp = psG.tile([D_H, 128], F32, tag="pg0")
        nc.tensor.transpose(tp[:], g_sb[:, j, :], identf[:])
        nc.vector.tensor_copy(gT[:, j * 128:(j + 1) * 128], tp[:])
    DC = D_H // 2
    mlp1 = psG.tile([DC, NH * 128], F32, tag="pg0")
    nc.tensor.matmul(mlp1[:], lhsT=w["wc1"][:], rhs=gT[:], start=True, stop=True)
    z = sb.tile([DC, NH * 128], F32, tag="z")
    nc.scalar.activation(z[:], mlp1[:], Relu, bias=w["bc1"][:])
    mlp2 = psG.tile([1, NH * 128], F32, tag="pg1")
    nc.tensor.matmul(mlp2[:], lhsT=w["wc2"][:], rhs=z[:], start=True, stop=True)
    o_sb = sb.tile([1, NH * 128], F32, tag="osb")
    nc.vector.tensor_scalar(out=o_sb[:], in0=mlp2[:], scalar1=w["bc2"][:],
                            scalar2=None, op0=mybir.AluOpType.add)
    nc.sync.dma_start(out_ap.rearrange("a b -> b a"), o_sb[:])

    for p in (dram, psG, psT, psR, psA, tmpp, hbp, o8p, msgp, sb):
        p.release()


# ---------------------------------------------------------------- compile+run

_CACHE = {}


def _compile(cfg):
    key = ("nc", cfg.t_bc, cfg.m_inst.tobytes())
    if key in _CACHE:
        return _CACHE[key]
    nc = bacc.Bacc("TRN2", target_bir_lowering=False, debug=False,
                   num_devices=N_CORES, num_swdge_queues=4)
    shapes = dict(
        xpad=([TBL_ROWS, 128], BF16),
        gidx=([128, cfg.n_inst, cfg.T * 8], I16),
        dloc=([128, NCH * cfg.ipb, cfg.T], BF16),
        iotaT=([128, 128 * cfg.T], BF16),
        iota256=([128, 256], BF16),
        identf=([128, 128], F32),
        invc=([128, NBLK], F32),
        batchv=([128, NBLK], F32),
        padmask=([128, 1], F32),
        xfull=([128, NBLK, D_IN], F32),
        w1le=([D_IN + 1, D_H], F32), w2le=([D_H + 1, D_H], F32),
        w3le=([D_H + 1, D_H], F32),
        w1r=([D_IN, D_H], F32), w2r=([D_H, D_H], F32), w3r=([D_H, D_H], F32),
        wc1=([D_H, D_H // 2], F32), wc2=([D_H // 2, 1], F32),
        bc1=([D_H // 2, 1], F32), bc2=([1, 1], F32),
    )
    ins = {}
    for name, (shp, dt) in shapes.items():
        ins[name] = nc.dram_tensor(name, shp, dt, kind="ExternalInput").ap()
    out = nc.dram_tensor("out", [N_GRAPHS, 1], F32, kind="ExternalOutput")
    with tile.TileContext(nc) as tc:
        build_gnn(tc, out.ap(), ins, cfg)
    nc.compile()
    _CACHE[key] = nc
    return nc


def make_in_maps(inputs):
    cfg, shared, per_core = build_host_data(
        inputs["x"], inputs["edge_index"], inputs["batch"])
    wmap = weight_inputs(
        inputs["W1l"], inputs["b1"], inputs["W1r"], inputs["W2l"], inputs["b2"],
        inputs["W2r"], inputs["W3l"], inputs["b3"], inputs["W3r"],
        inputs["Wc1"], inputs["bc1"], inputs["Wc2"], inputs["bc2"])
    in_maps = []
    for c in range(N_CORES):
        m = {}
        m.update(shared)
        m.update(per_core[c])
        m.update(wmap)
        in_maps.append(m)
    return cfg, in_maps


def _make_executor(nc):
    """Build a reusable jitted 8-core executor for the compiled Bass module."""
    import jax
    from jax.sharding import Mesh, PartitionSpec
    from jax.experimental.shard_map import shard_map
    from concourse.bass2jax import (_bass_exec_p, install_neuronx_cc_hook,
                                    partition_id_tensor)
    install_neuronx_cc_hook()
    partition_name = (nc.partition_id_tensor.name
                      if nc.partition_id_tensor else None)
    in_names, out_names, out_avals = [], [], []
    for alloc in nc.m.functions[0].allocations:
        if not isinstance(alloc, mybir.MemoryLocationSet):
            continue
        name = alloc.memorylocations[0].name
        if alloc.kind == "ExternalInput":
            if name != partition_name:
                in_names.append(name)
        elif alloc.kind == "ExternalOutput":
            out_names.append(name)
            out_avals.append(jax.core.ShapedArray(
                tuple(alloc.tensor_shape), mybir.dt.np(alloc.dtype)))
    n_params = len(in_names)
    in_names_all = list(in_names) + list(out_names)
    if partition_name:
        in_names_all.append(partition_name)

    def _body(*args):
        operands = list(args)
        if partition_name:
            operands.append(partition_id_tensor())
        return tuple(_bass_exec_p.bind(
            *operands, out_avals=tuple(out_avals),
            in_names=tuple(in_names_all), out_names=tuple(out_names),
            lowering_input_output_aliases=(), sim_require_finite=False,
            sim_require_nnan=False, nc=nc))

    devices = jax.devices()[:N_CORES]
    mesh = Mesh(np.asarray(devices), ("core",))
    n_outs = len(out_names)
    sharded = jax.jit(shard_map(
        _body, mesh=mesh,
        in_specs=(PartitionSpec("core"),) * (n_params + n_outs),
        out_specs=(PartitionSpec("core"),) * n_outs, check_rep=False),
        keep_unused=True)

    def run(in_maps):
        concat_in = [np.concatenate([np.asarray(in_maps[c][n])
                                     for c in range(N_CORES)], axis=0)
                     for n in in_names]
        concat_zeros = [np.zeros((N_CORES * a.shape[0], *a.shape[1:]), a.dtype)
                        for a in out_avals]
        args = [jax.device_put(a) for a in concat_in + concat_zeros]
        out_arrs = sharded(*args)
        jax.block_until_ready(out_arrs)
        return {name: np.asarray(out_arrs[i]).reshape(
                    N_CORES, *out_avals[i].shape)[0]
                for i, name in enumerate(out_names)}, (args, sharded)
    return run


def _get_runner(cfg):
    key = ("runner", cfg.t_bc, cfg.m_inst.tobytes())
    if key not in _CACHE:
        _CACHE[key] = _make_executor(_compile(cfg))
    return _CACHE[key]


def kernel(**inputs):
    cfg, in_maps = make_in_maps(inputs)
    run = _get_runner(cfg)
    out, _ = run(in_maps)
    return np.asarray(out["out"], np.float32)

